# revision 3
# baseline (speedup 1.0000x reference)
"""Trainium2 Bass kernel for the JointLoss problem (contrastive NT-Xent + 2 MSE terms).

kernel(representation, xrecon, xorig) -> (loss, closs, recon_loss, zrecon_loss)

Strategy (8 NeuronCores, SPMD — one NEFF, per-core variation only via inputs):
  - closs: sim = z@z.T with z = r/||r||.  Fold the normalization and 1/tau into
    per-column scales s_j = 1/(||r_j|| sqrt(tau)) applied to R^T before the GEMM:
    each core computes a (512, 4096) slab of logits = (raw R^T columns).T @ (scaled R^T),
    applies the per-row scale s_m inside the fused exp (activation scale AP), and
    accumulates row sums with the activation accumulator.  Positives come from the
    diagonal of the partner block; the self-similarity term is the constant e^(1/tau).
  - Column chunks of R^T are permuted per core so chunk0 = partner block and
    chunk1 = own slab, making the kernel core-id independent.
  - recon/zrecon MSE partials: row-sharded subtract + Square with accumulator.
  - All partials reduced over partitions with one fp32 matmul -> (10,1) output/core;
    host sums the 8 cores' partials.
"""

import math

import ml_dtypes
import numpy as np

TAU = 0.5
N = 2048
TWO_N = 4096
D = 512
NCORES = 8
CH = 512  # column chunk (one per core-slab)

_CACHE = {}


def _build_nc():
    import concourse.bacc as bacc
    import concourse.mybir as mybir
    import concourse.tile as tile
    from concourse.masks import make_identity

    F32 = mybir.dt.float32
    BF16 = mybir.dt.bfloat16
    AX = mybir.AxisListType
    OP = mybir.AluOpType
    AF = mybir.ActivationFunctionType

    nc = bacc.Bacc("TRN2", target_bir_lowering=False, debug=False)
    rt = nc.dram_tensor("rt", [D, TWO_N], BF16, kind="ExternalInput")
    xr = nc.dram_tensor("xr", [CH, 1024], F32, kind="ExternalInput")
    xo = nc.dram_tensor("xo", [CH, 1024], F32, kind="ExternalInput")
    zi = nc.dram_tensor("zi", [256, D], F32, kind="ExternalInput")
    zj = nc.dram_tensor("zj", [256, D], F32, kind="ExternalInput")
    out = nc.dram_tensor("out", [10, 1], F32, kind="ExternalOutput")

    EXP_DIAG = math.exp(1.0 / TAU)

    with tile.TileContext(nc) as tc:
        with (
            tc.tile_pool(name="singles", bufs=1) as singles,
            tc.tile_pool(name="sqp", bufs=8) as sqp,
            tc.tile_pool(name="srowp", bufs=2) as srowp,
            tc.tile_pool(name="msep", bufs=2) as msep,
            tc.tile_pool(name="smallp", bufs=4) as smallp,
            tc.tile_pool(name="mpsum", bufs=2, space="PSUM") as mpsum,
            tc.tile_pool(name="spsum", bufs=1, space="PSUM") as spsum,
            tc.tile_pool(name="bpsum", bufs=1, space="PSUM") as bpsum,
            tc.tile_pool(name="tpsum", bufs=1, space="PSUM") as tpsum,
        ):
            ident = singles.tile([128, 128], F32, tag="ident")
            make_identity(nc, ident)
            ones_k = singles.tile([128, 1], BF16, tag="ones_k")
            nc.vector.memset(ones_k, 1.0)
            ones1 = singles.tile([1, 128], BF16, tag="ones1")
            nc.vector.memset(ones1, 1.0)
            ones_f = singles.tile([128, 1], F32, tag="ones_f")
            nc.vector.memset(ones_f, 1.0)
            negdiag = singles.tile([128, 1], F32, tag="negdiag")
            nc.vector.memset(negdiag, -EXP_DIAG)
            s_bcast = singles.tile([128, TWO_N], BF16, tag="s_bcast")
            eacc = singles.tile([128, 16], F32, tag="eacc")
            stats = singles.tile([128, 10], F32, tag="stats")
            smat = singles.tile([128, 4], F32, tag="smat")
            poslog = singles.tile([128, 4], F32, tag="poslog")

            rt_p = {}
            rts_p = {}
            sq_p = {}

            def prep(cc):
                ccp, half = cc // 2, cc % 2
                if half == 0:
                    for d in range(4):
                        t = singles.tile([128, 1024], BF16, tag=f"rt_{d}_{ccp}")
                        nc.sync.dma_start(
                            t, rt[128 * d : 128 * (d + 1), 1024 * ccp : 1024 * (ccp + 1)]
                        )
                        rt_p[(d, ccp)] = t
                        s = sqp.tile([128, 1024], BF16, tag="sq")
                        nc.vector.tensor_tensor(s, t, t, OP.mult)
                        sq_p[(d, ccp)] = s
                # column sum-of-squares for this 512-chunk -> [1, 512]
                ps = spsum.tile([1, CH], F32, tag="psum_s")
                for d in range(4):
                    nc.tensor.matmul(
                        ps,
                        ones_k,
                        sq_p[(d, ccp)][:, CH * half : CH * (half + 1)],
                        start=(d == 0),
                        stop=(d == 3),
                    )
                # s = exp(-0.5 * ln(tau * sumsq)) = 1/(sqrt(tau)*||r||)
                lnt = smallp.tile([1, CH], F32, tag="lnt")
                nc.scalar.activation(lnt, ps, AF.Ln, scale=TAU)
                srow = srowp.tile([1, CH], BF16, tag="srow")
                nc.scalar.activation(srow, lnt, AF.Exp, scale=-0.5)
                # broadcast to all 128 partitions via K=1 matmul
                pb = bpsum.tile([128, CH], F32, tag="psum_b")
                nc.tensor.matmul(pb, ones1, srow, start=True, stop=True)
                nc.scalar.copy(s_bcast[:, CH * cc : CH * (cc + 1)], pb)
                if cc == 1:
                    # per-slab-row scales (own chunk lives at permuted cols 512..1023)
                    psm = tpsum.tile([128, 4], F32, tag="psum_sm")
                    for rr in range(4):
                        for d in range(4):
                            nc.tensor.matmul(
                                psm[:, rr : rr + 1],
                                sq_p[(d, 0)][:, 512 + 128 * rr : 512 + 128 * (rr + 1)],
                                ones_k,
                                start=(d == 0),
                                stop=(d == 3),
                            )
                    lnm = smallp.tile([128, 4], F32, tag="lnm")
                    nc.scalar.activation(lnm, psm, AF.Ln, scale=TAU)
                    nc.scalar.activation(smat, lnm, AF.Exp, scale=-0.5)
                if half == 1:
                    for d in range(4):
                        t2 = singles.tile([128, 1024], BF16, tag=f"rts_{d}_{ccp}")
                        nc.vector.tensor_tensor(
                            t2,
                            rt_p[(d, ccp)],
                            s_bcast[:, 1024 * ccp : 1024 * (ccp + 1)],
                            OP.mult,
                        )
                        rts_p[(d, ccp)] = t2

            def main_block(ccp):
                for rr in range(4):
                    ps = mpsum.tile([128, 1024], F32, tag="mps")
                    for half in range(2):
                        for d in range(4):
                            nc.tensor.matmul(
                                ps[:, CH * half : CH * (half + 1)],
                                rt_p[(d, 0)][:, 512 + 128 * rr : 512 + 128 * (rr + 1)],
                                rts_p[(d, ccp)][:, CH * half : CH * (half + 1)],
                                start=(d == 0),
                                stop=(d == 3),
                            )
                    if ccp == 0:
                        # positives: diagonal of the partner block (permuted cols 0..511)
                        ext = smallp.tile([128, 128], F32, tag="ext")
                        nc.vector.tensor_tensor(
                            ext, ps[:, 128 * rr : 128 * (rr + 1)], ident, OP.mult
                        )
                        posr = smallp.tile([128, 1], F32, tag="posr")
                        nc.vector.reduce_sum(posr, ext, axis=AX.X)
                        nc.vector.tensor_tensor(
                            poslog[:, rr : rr + 1], posr, smat[:, rr : rr + 1], OP.mult
                        )
                    nc.scalar.activation(
                        ps,
                        ps,
                        AF.Exp,
                        scale=smat[:, rr : rr + 1],
                        accum_out=eacc[:, 4 * rr + ccp : 4 * rr + ccp + 1],
                    )

            for ccp in range(4):
                prep(2 * ccp)
                prep(2 * ccp + 1)
                main_block(ccp)

            # MSE partials
            for t in range(4):
                xrt = msep.tile([128, 1024], F32, tag="xrt")
                nc.sync.dma_start(xrt, xr[128 * t : 128 * (t + 1), :])
                xot = msep.tile([128, 1024], F32, tag="xot")
                nc.sync.dma_start(xot, xo[128 * t : 128 * (t + 1), :])
                dx = msep.tile([128, 1024], F32, tag="dx")
                nc.vector.tensor_tensor(dx, xrt, xot, OP.subtract)
                trash = msep.tile([128, 1024], BF16, tag="trashx")
                nc.scalar.activation(
                    trash, dx, AF.Square, accum_out=stats[:, 4 + t : 5 + t]
                )
            for t in range(2):
                zit = msep.tile([128, D], F32, tag="zit")
                nc.sync.dma_start(zit, zi[128 * t : 128 * (t + 1), :])
                zjt = msep.tile([128, D], F32, tag="zjt")
                nc.sync.dma_start(zjt, zj[128 * t : 128 * (t + 1), :])
                dz = msep.tile([128, D], F32, tag="dz")
                nc.vector.tensor_tensor(dz, zit, zjt, OP.subtract)
                trz = msep.tile([128, D], BF16, tag="trashz")
                nc.scalar.activation(
                    trz, dz, AF.Square, accum_out=stats[:, 8 + t : 9 + t]
                )

            # per-row loss: ln(rowsum_exp - e^(1/tau)) - pos_logit
            for rr in range(4):
                rsv = smallp.tile([128, 1], F32, tag="rsv")
                nc.vector.tensor_reduce(
                    rsv, eacc[:, 4 * rr : 4 * rr + 4], axis=AX.X, op=OP.add
                )
                lnr = smallp.tile([128, 1], F32, tag="lnr")
                nc.scalar.activation(lnr, rsv, AF.Ln, bias=negdiag)
                nc.vector.tensor_tensor(
                    stats[:, rr : rr + 1], lnr, poslog[:, rr : rr + 1], OP.subtract
                )

            # partition-reduce all partials with one fp32 matmul
            po = tpsum.tile([10, 1], F32, tag="po")
            nc.tensor.matmul(po, stats, ones_f, start=True, stop=True)
            osb = smallp.tile([10, 1], F32, tag="osb")
            nc.scalar.copy(osb, po)
            nc.sync.dma_start(out[:, :], osb)

    nc.compile()
    return nc


def _get_nc():
    if "nc" not in _CACHE:
        _CACHE["nc"] = _build_nc()
    return _CACHE["nc"]


def make_in_maps(representation, xrecon, xorig):
    rep = np.ascontiguousarray(np.asarray(representation, dtype=np.float32))
    xrec = np.asarray(xrecon, dtype=np.float32)
    xorg = np.asarray(xorig, dtype=np.float32)
    RT = np.ascontiguousarray(rep.T).astype(ml_dtypes.bfloat16)  # (512, 4096)
    in_maps = []
    for c in range(NCORES):
        partner = (c + 4) % 8
        order = [partner, c] + sorted(set(range(8)) - {partner, c})
        rt_c = np.concatenate([RT[:, CH * p : CH * (p + 1)] for p in order], axis=1)
        in_maps.append(
            {
                "rt": np.ascontiguousarray(rt_c),
                "xr": np.ascontiguousarray(xrec[CH * c : CH * (c + 1)]),
                "xo": np.ascontiguousarray(xorg[CH * c : CH * (c + 1)]),
                "zi": np.ascontiguousarray(rep[256 * c : 256 * (c + 1)]),
                "zj": np.ascontiguousarray(rep[2048 + 256 * c : 2048 + 256 * (c + 1)]),
            }
        )
    return in_maps


def combine_outputs(per_core_out):
    """per_core_out: list of 8 arrays shaped (10,1) float32."""
    r = np.stack([np.asarray(o).reshape(10) for o in per_core_out]).astype(np.float64)
    closs = r[:, 0:4].sum() / TWO_N
    recon = r[:, 4:8].sum() / TWO_N
    zrec = r[:, 8:10].sum() / N
    loss = recon + closs + zrec
    f = np.float32
    return (f(loss), f(closs), f(recon), f(zrec))


def kernel(representation, xrecon, xorig):
    from concourse.bass_utils import run_bass_kernel_spmd

    nc = _get_nc()
    in_maps = make_in_maps(representation, xrecon, xorig)
    res = run_bass_kernel_spmd(nc, in_maps, core_ids=list(range(NCORES)))
    return combine_outputs([res.results[c]["out"] for c in range(NCORES)])
